# revision 38
# baseline (speedup 1.0000x reference)
"""LMClassifier forward (mean masked cross-entropy) on 8 Trainium2 cores.

Ragged-aware sharding: the reference only counts positions t < len_b - 2,
and the masked-mean CE is order-invariant, so the host compacts the
~N_valid valid (t, b) tokens into a flat list and deals them across all
8 cores (token-parallel, FULL vocab per core — no vocab split, so emb is
computed once per token and each core returns a complete sumexp).

Per core (NT tokens = ceil(N_valid / 8) padded to 128):
  emb = sigmoid(ctx @ W1.T + b1)                     [NT, E]
  sumexp[tok] = sum_v exp((emb @ W2.T + b2) * it)    full (padded) vocab
  tgt_raw[tok] = emb . W2[tgt[tok]]                  (ones-matmul reduce)
Host assembles mean NLL from sumexp / tgt_raw of the real tokens only.
"""

import contextlib

import numpy as np
import ml_dtypes

import concourse.bacc as bacc
import concourse.tile as tile
import concourse.mybir as mybir
from concourse.bass_utils import run_bass_kernel_spmd

BF16 = mybir.dt.bfloat16
FP32 = mybir.dt.float32
AF = mybir.ActivationFunctionType

FP8 = mybir.dt.float8e4
FP8NP = mybir.dt.np(mybir.dt.float8e4)
W2_SCALE = 64.0  # keeps fp8-cast W2 out of the denormal range
W1_SCALE = 64.0  # same for W1; sigmoid's free affine divides it back out

T, B, H, E, V = 256, 32, 2048, 1024, 50257
NC = 8                      # all 8 cores token-parallel
VC = 50688                  # 49*1024 + 512 tail; 431 zero-padded vocab cols
PAD_V = VC - V


def _chunks(total, size):
    out, off = [], 0
    while off < total:
        n = min(size, total - off)
        out.append((off, n))
        off += n
    return out


class Cfg:
    def __init__(self, H, E, NT, VC, inv_temp=1.0, use_b2=False, use_b1=False):
        assert H % 128 == 0 and E % 128 == 0 and NT % 128 == 0
        assert VC % 512 == 0
        self.H, self.E, self.NT, self.VC = H, E, NT, VC
        self.inv_temp = float(inv_temp)
        self.use_b2 = use_b2
        self.use_b1 = use_b1
        self.n_k = H // 128      # contraction tiles for matmul1
        self.n_e = E // 128      # e-blocks (contraction tiles for matmul2)
        self.n_sub = NT // 128   # token subblocks
        self.tblocks = _chunks(NT, 512)   # phase-A token blocks
        self.vblocks = _chunks(VC, 1024)  # phase-B vocab blocks (512 tail ok)
        self.n_vb = len(self.vblocks)
        assert self.n_e % 2 == 0 and self.n_k % 2 == 0


def build_lm_program(cfg):
    """Build the per-core SPMD Bass program. Returns compiled nc."""
    H, E, NT, VC = cfg.H, cfg.E, cfg.NT, cfg.VC
    n_k, n_e, n_sub, n_vb = cfg.n_k, cfg.n_e, cfg.n_sub, cfg.n_vb
    nc = bacc.Bacc("TRN2", debug=False, target_bir_lowering=False)

    ctxT = nc.dram_tensor("ctxT", [H, NT], FP8, kind="ExternalInput").ap()
    # w1t layout: [p, e*n_k + k, ec] = W1[e*128+ec, k*128+p] (host pre-shuffled
    # so each e-block is one contiguous-per-partition 2KB DMA)
    w1t = nc.dram_tensor("w1t", [128, (H // 128) * E], FP8, kind="ExternalInput").ap()
    w2t = nc.dram_tensor("w2t", [E, VC], FP8, kind="ExternalInput").ap()
    w2tgtT = nc.dram_tensor("w2tgtT", [E, NT], BF16, kind="ExternalInput").ap()
    if cfg.use_b1:
        b1 = nc.dram_tensor("b1", [E, 1], FP32, kind="ExternalInput").ap()
    if cfg.use_b2:
        b2row = nc.dram_tensor("b2row", [1, VC], FP32, kind="ExternalInput").ap()
    sumexp_out = nc.dram_tensor(
        "sumexp_out", [128, n_sub], FP32, kind="ExternalOutput"
    ).ap()
    tgt_out = nc.dram_tensor("tgt_out", [1, NT], FP32, kind="ExternalOutput").ap()

    with contextlib.ExitStack() as ex:
        tc = ex.enter_context(tile.TileContext(nc))
        # few pools: each pool costs per-engine barrier/sem teardown at exit
        persist_pool = ex.enter_context(tc.tile_pool(name="persist", bufs=1))
        stream_pool = ex.enter_context(tc.tile_pool(name="stream", bufs=2))
        w2_pool = ex.enter_context(tc.tile_pool(name="w2", bufs=4))
        psa_pool = ex.enter_context(tc.tile_pool(name="psa", bufs=3, space="PSUM"))
        pstp_pool = ex.enter_context(tc.tile_pool(name="pstp", bufs=1, space="PSUM"))
        ps2_pool = ex.enter_context(tc.tile_pool(name="ps2", bufs=2, space="PSUM"))
        const_pool = w1_pool = emb_pool = acc_pool = persist_pool
        ctx_pool = tgtw_pool = tmp_pool = stream_pool
        ps1_pool = psa_pool
        pst_pool = pstp_pool

        w2v = w2t.rearrange("(e p) v -> p e v", p=128)
        tgv = w2tgtT.rearrange("(e p) t -> p e t", p=128)
        ctxv = ctxT.rearrange("(k p) t -> p k t", p=128)

        def issue_ctx(bi):
            # two independent tiles so the two halves transfer concurrently
            # (multiple DMAs into one tile serialize on write-after-write)
            bs, bn = cfg.tblocks[bi]
            hk = n_k // 2
            ta = ctx_pool.tile([128, hk, 512], FP8, tag="ctxa")
            tb = ctx_pool.tile([128, hk, 512], FP8, tag="ctxb")
            nc.sync.dma_start(ta[:, :, :bn], ctxv[:, :hk, bs : bs + bn])
            nc.scalar.dma_start(tb[:, :, :bn], ctxv[:, hk:, bs : bs + bn])
            return ta, tb

        def issue_tgw(bi):
            bs, bn = cfg.tblocks[bi]
            t = tgtw_pool.tile([128, n_e * 512], BF16, tag="tgw")
            nc.gpsimd.dma_start(t[:, : n_e * bn], tgv[:, :, bs : bs + bn])
            return t

        def issue_w2(vi):
            vo, vn = cfg.vblocks[vi]
            t = w2_pool.tile([128, n_e, 1024], FP8, tag="w2s")
            nc.sync.dma_start(t[:, :, :vn], w2v[:, :, vo : vo + vn])
            return t

        # ---- PE warm-up: junk matmuls on a memset tile so the HAM clock
        # gate is at 8/8 by the time the real (DMA-gated) matmuls start ----
        WRM = stream_pool.tile([128, 512], BF16, tag="warm")
        nc.vector.memset(WRM[:, :], 0.0)
        psw = ps1_pool.tile([128, 512], FP32, tag="ps1")
        for i in range(14):
            nc.tensor.matmul(psw[:, :], WRM[:, :128], WRM[:, :], start=True, stop=True)

        # ---- upfront DMA issue, in consumption order: only ~8 DMA
        # completion-sem lanes exist and a lane frees when the consumer
        # instruction runs, so issue what phase A needs first and keep
        # everything else off the two HWDGE engines (memset / gpsimd)
        ONES = const_pool.tile([128, 1], BF16, tag="ones")
        nc.gpsimd.memset(ONES[:, :], 1.0)
        B1S = const_pool.tile([128, n_e], FP32, tag="b1s")
        if cfg.use_b1:
            nc.gpsimd.dma_start(B1S[:, :], b1.rearrange("(e p) one -> p (e one)", p=128))
        else:
            nc.gpsimd.memset(B1S[:, :], 0.0)
        ctx_tiles = {0: issue_ctx(0)}
        w1_tiles = []
        for e in range(n_e):
            t = w1_pool.tile([128, n_k, 128], FP8, tag=f"w1s{e}")
            eng = nc.sync if e % 2 == 0 else nc.scalar
            eng.dma_start(t[:, :, :], w1t[:, e * n_k * 128 : (e + 1) * n_k * 128])
            w1_tiles.append(t)
        tgw_tiles = {0: issue_tgw(0)}
        if len(cfg.tblocks) > 1:
            ctx_tiles[1] = issue_ctx(1)
        if cfg.use_b2:
            B2S = const_pool.tile([1, VC], FP32, tag="b2s")
            nc.gpsimd.dma_start(B2S[:, :], b2row[:, :])
            ONE1 = const_pool.tile([1, 128], FP32, tag="one1")
            nc.any.memset(ONE1[:, :], 1.0)
        w2_tiles = {0: issue_w2(0)}
        if n_vb > 1:
            w2_tiles[1] = issue_w2(1)
        WRMS = stream_pool.tile([128, 1], BF16, tag="warms")
        nc.scalar.activation(WRMS[:, :], WRM[:, :1], AF.Sigmoid)

        EMB8 = emb_pool.tile([128, n_e, NT], FP8, tag="emb8")
        SUMP = acc_pool.tile([128, n_sub * n_vb], FP32, tag="sump")
        SOUT = acc_pool.tile([128, n_sub], FP32, tag="sout")
        TGT = acc_pool.tile([1, NT], FP32, tag="tgt")

        # ---- phase A: emb = sigmoid(W1 @ ctx + b1), [e, t] layout ----
        sig_scale = 1.0 / W1_SCALE
        hk2 = n_k // 4  # kp pairs per ctx half
        for bi, (bs, bn) in enumerate(cfg.tblocks):
            CTA, CTB = ctx_tiles.pop(bi) if bi in ctx_tiles else issue_ctx(bi)
            for e in range(n_e):
                ps1 = ps1_pool.tile([128, 512], FP32, tag="ps1")
                for kp in range(n_k // 2):
                    ct = CTA if kp < hk2 else CTB
                    ko = 2 * kp if kp < hk2 else 2 * (kp - hk2)
                    nc.tensor.matmul(
                        ps1[:, :bn],
                        w1_tiles[e][:, 2 * kp : 2 * kp + 2, :],
                        ct[:, ko : ko + 2, :bn],
                        start=(kp == 0),
                        stop=(kp == n_k // 2 - 1),
                        perf_mode=mybir.MatmulPerfMode.DoubleRow,
                    )
                nc.scalar.activation(
                    EMB8[:, e : e + 1, bs : bs + bn],
                    ps1[:, :bn],
                    AF.Sigmoid,
                    bias=B1S[:, e : e + 1],
                    scale=sig_scale,
                )
            # ---- phase A2: tgt_raw for this block ----
            TGW = tgw_tiles.pop(bi) if bi in tgw_tiles else issue_tgw(bi)
            if bi + 2 < len(cfg.tblocks):
                ctx_tiles[bi + 2] = issue_ctx(bi + 2)
            pst = pst_pool.tile([1, 512], FP32, tag="pst")
            for e in range(n_e):
                tmp = tmp_pool.tile([128, 512], BF16, tag="tmp")
                eng = nc.vector if e % 2 == 0 else nc.gpsimd
                eng.tensor_mul(
                    tmp[:, :bn],
                    EMB8[:, e, bs : bs + bn],
                    TGW[:, e * bn : (e + 1) * bn],
                )
                nc.tensor.matmul(
                    pst[:, :bn],
                    ONES[:, :],
                    tmp[:, :bn],
                    start=(e == 0),
                    stop=(e == n_e - 1),
                )
            nc.vector.tensor_copy(TGT[:, bs : bs + bn], pst[:, :bn])
        nc.gpsimd.dma_start(tgt_out[:, :], TGT[:, :])

        # ---- phase B: logits, exp, accumulate ----
        exp_scale = cfg.inv_temp / W2_SCALE
        for vi, (vo, vn) in enumerate(cfg.vblocks):
            W2S8 = w2_tiles.pop(vi) if vi in w2_tiles else issue_w2(vi)
            if vi == 0:
                for j in (2, 3):
                    if j < n_vb:
                        w2_tiles[j] = issue_w2(j)
            elif vi + 3 < n_vb:
                w2_tiles[vi + 3] = issue_w2(vi + 3)
            for sub in range(n_sub):
                ps2 = ps2_pool.tile([128, 1024], FP32, tag="ps2")
                for ep in range(n_e // 2):
                    lhsT = EMB8[:, 2 * ep : 2 * ep + 2, sub * 128 : (sub + 1) * 128]
                    for h in range(vn // 512):
                        nc.tensor.matmul(
                            ps2[:, h * 512 : (h + 1) * 512],
                            lhsT,
                            W2S8[:, 2 * ep : 2 * ep + 2, h * 512 : (h + 1) * 512],
                            start=(ep == 0),
                            stop=(ep == n_e // 2 - 1) and not cfg.use_b2,
                            perf_mode=mybir.MatmulPerfMode.DoubleRow,
                        )
                if cfg.use_b2:
                    for h in range(vn // 512):
                        nc.tensor.matmul(
                            ps2[:, h * 512 : (h + 1) * 512],
                            ONE1[:, :],
                            B2S[:, vo + h * 512 : vo + (h + 1) * 512],
                            start=False,
                            stop=True,
                        )
                nc.scalar.activation(
                    ps2[:, :vn],
                    ps2[:, :vn],
                    AF.Exp,
                    scale=exp_scale,
                    accum_out=SUMP[:, sub * n_vb + vi : sub * n_vb + vi + 1],
                )

        # ---- phase C: reduce partials, write outputs ----
        for sub in range(n_sub):
            nc.vector.reduce_sum(
                SOUT[:, sub : sub + 1],
                SUMP[:, sub * n_vb : (sub + 1) * n_vb],
                axis=mybir.AxisListType.X,
            )
        nc.sync.dma_start(sumexp_out[:, :], SOUT[:, :])

    nc.compile()
    return nc


# ---------------- host side ----------------


def _plan(lens):
    """Compact valid (b-major) token indices; deal them across NC cores."""
    lens = np.asarray(lens, np.int64)
    nv = np.clip(lens - 2, 0, T - 2)
    n_valid = int(nv.sum())
    NT = max(128, 128 * int(np.ceil(n_valid / (NC * 128))))
    base = np.arange(T - 2, dtype=np.int64)
    idx_all = np.concatenate([b * (T - 2) + base[: nv[b]] for b in range(B)])
    idx = np.zeros(NC * NT, np.int64)
    idx[:n_valid] = idx_all
    idx = idx.reshape(NC, NT)
    counts = [max(0, min(NT, n_valid - c * NT)) for c in range(NC)]
    return NT, idx, counts


def _shard_inputs(hidden, token, W1, W2, NT, idx, b1=None):
    bf16 = ml_dtypes.bfloat16
    half = H // 2
    ctx = np.concatenate(
        [hidden[: T - 2, :, :half], hidden[2:, :, half:]], axis=-1
    )  # [T-2, B, H]
    ctxT8 = np.transpose(ctx, (2, 1, 0)).reshape(H, B * (T - 2)).astype(FP8NP)
    # [p, e, k, ec] = W1[e*128+ec, k*128+p] -> flat [128, n_k*E]
    n_e, n_k = E // 128, H // 128
    W1T = np.ascontiguousarray(
        (W1.reshape(n_e, 128, n_k, 128) * W1_SCALE).transpose(3, 0, 2, 1)
    ).reshape(128, n_k * E).astype(FP8NP)
    W2T = np.zeros((E, VC), dtype=FP8NP)
    W2T[:, :V] = (W2.T * W2_SCALE).astype(FP8NP)
    tgtf = np.ascontiguousarray(token[1 : T - 1].T).reshape(B * (T - 2))

    in_maps, tgt_ids = [], []
    for c in range(NC):
        ic = idx[c]
        tg = tgtf[ic]  # [NT] target token id per slot
        w2gT = np.ascontiguousarray(W2[tg, :].T).astype(bf16)  # [E, NT]
        tgt_ids.append(tg)
        in_maps.append(
            dict(
                ctxT=np.ascontiguousarray(ctxT8[:, ic]),
                w1t=W1T,
                w2t=W2T,
                w2tgtT=w2gT,
            )
        )
    return in_maps, tgt_ids


def _combine(results, counts, tgt_ids, b2, inv_temp, NT):
    """results: list of NC dicts with sumexp_out [128, n_sub], tgt_out [1, NT]."""
    it = float(np.asarray(inv_temp).reshape(-1)[0])
    b2 = np.asarray(b2, dtype=np.float64)
    total_nll = 0.0
    total_cnt = 0
    for c in range(NC):
        k = counts[c]
        if k == 0:
            continue
        se = np.asarray(results[c]["sumexp_out"], dtype=np.float64)  # [128, n_sub]
        S = se.T.reshape(NT)[:k] - PAD_V  # exp(0)=1 per zero-padded vocab col
        raw = np.asarray(results[c]["tgt_out"], dtype=np.float64).reshape(NT)[:k]
        tg = tgt_ids[c][:k]
        logp_tgt = (raw + b2[tg]) * it - np.log(S)
        total_nll += -logp_tgt.sum()
        total_cnt += k
    return np.float32(total_nll / total_cnt)


def kernel(hidden, lens, token, W1, b1, W2, b2, inv_temp):
    hidden = np.asarray(hidden, dtype=np.float32)
    lens = np.asarray(lens, dtype=np.int32)
    token = np.asarray(token, dtype=np.int32)
    W1 = np.asarray(W1, dtype=np.float32)
    b1 = np.asarray(b1, dtype=np.float32)
    W2 = np.asarray(W2, dtype=np.float32)
    b2 = np.asarray(b2, dtype=np.float32)
    inv_temp = np.asarray(inv_temp, dtype=np.float32)

    use_b2 = bool(np.any(b2 != 0.0))
    use_b1 = bool(np.any(b1 != 0.0))
    NT, idx, counts = _plan(lens)
    cfg = Cfg(H, E, NT, VC, inv_temp=float(inv_temp.reshape(-1)[0]), use_b2=use_b2,
              use_b1=use_b1)
    nc = build_lm_program(cfg)
    in_maps, tgt_ids = _shard_inputs(hidden, token, W1, W2, NT, idx,
                                     b1=b1 if use_b1 else None)
    if use_b1:
        b1c = np.ascontiguousarray(np.asarray(b1).reshape(E, 1)).astype(np.float32)
        for c in range(NC):
            in_maps[c]["b1"] = b1c
    if use_b2:
        b2p = np.zeros((1, VC), dtype=np.float32)
        b2p[0, :V] = b2 * W2_SCALE
        for c in range(NC):
            in_maps[c]["b2row"] = b2p
    res = run_bass_kernel_spmd(nc, in_maps, core_ids=list(range(NC)))
    return _combine(res.results, counts, tgt_ids, b2, inv_temp, NT)


# revision 39
# speedup vs baseline: 1.0003x; 1.0003x over previous
"""LMClassifier forward (mean masked cross-entropy) on 8 Trainium2 cores.

Ragged-aware sharding: the reference only counts positions t < len_b - 2,
and the masked-mean CE is order-invariant, so the host compacts the
~N_valid valid (t, b) tokens into a flat list and deals them across all
8 cores (token-parallel, FULL vocab per core — no vocab split, so emb is
computed once per token and each core returns a complete sumexp).

Per core (NT tokens = ceil(N_valid / 8) padded to 128):
  emb = sigmoid(ctx @ W1.T + b1)                     [NT, E]
  sumexp[tok] = sum_v exp((emb @ W2.T + b2) * it)    full (padded) vocab
  tgt_raw[tok] = emb . W2[tgt[tok]]                  (ones-matmul reduce)
Host assembles mean NLL from sumexp / tgt_raw of the real tokens only.
"""

import contextlib

import numpy as np
import ml_dtypes

import concourse.bacc as bacc
import concourse.tile as tile
import concourse.mybir as mybir
from concourse.bass_utils import run_bass_kernel_spmd

BF16 = mybir.dt.bfloat16
FP32 = mybir.dt.float32
AF = mybir.ActivationFunctionType

FP8 = mybir.dt.float8e4
FP8NP = mybir.dt.np(mybir.dt.float8e4)
W2_SCALE = 64.0  # keeps fp8-cast W2 out of the denormal range
W1_SCALE = 64.0  # same for W1; sigmoid's free affine divides it back out

T, B, H, E, V = 256, 32, 2048, 1024, 50257
NC = 8                      # all 8 cores token-parallel
VC = 50688                  # 49*1024 + 512 tail; 431 zero-padded vocab cols
PAD_V = VC - V


def _chunks(total, size):
    out, off = [], 0
    while off < total:
        n = min(size, total - off)
        out.append((off, n))
        off += n
    return out


class Cfg:
    def __init__(self, H, E, NT, VC, inv_temp=1.0, use_b2=False, use_b1=False):
        assert H % 128 == 0 and E % 128 == 0 and NT % 128 == 0
        assert VC % 512 == 0
        self.H, self.E, self.NT, self.VC = H, E, NT, VC
        self.inv_temp = float(inv_temp)
        self.use_b2 = use_b2
        self.use_b1 = use_b1
        self.n_k = H // 128      # contraction tiles for matmul1
        self.n_e = E // 128      # e-blocks (contraction tiles for matmul2)
        self.n_sub = NT // 128   # token subblocks
        self.tblocks = _chunks(NT, 512)   # phase-A token blocks
        self.vblocks = _chunks(VC, 1024)  # phase-B vocab blocks (512 tail ok)
        self.n_vb = len(self.vblocks)
        assert self.n_e % 2 == 0 and self.n_k % 2 == 0


def build_lm_program(cfg):
    """Build the per-core SPMD Bass program. Returns compiled nc."""
    H, E, NT, VC = cfg.H, cfg.E, cfg.NT, cfg.VC
    n_k, n_e, n_sub, n_vb = cfg.n_k, cfg.n_e, cfg.n_sub, cfg.n_vb
    nc = bacc.Bacc("TRN2", debug=False, target_bir_lowering=False)

    ctxT = nc.dram_tensor("ctxT", [H, NT], FP8, kind="ExternalInput").ap()
    # w1t layout: [p, e*n_k + k, ec] = W1[e*128+ec, k*128+p] (host pre-shuffled
    # so each e-block is one contiguous-per-partition 2KB DMA)
    w1t = nc.dram_tensor("w1t", [128, (H // 128) * E], FP8, kind="ExternalInput").ap()
    w2t = nc.dram_tensor("w2t", [E, VC], FP8, kind="ExternalInput").ap()
    w2tgtT = nc.dram_tensor("w2tgtT", [E, NT], BF16, kind="ExternalInput").ap()
    if cfg.use_b1:
        b1 = nc.dram_tensor("b1", [E, 1], FP32, kind="ExternalInput").ap()
    if cfg.use_b2:
        b2row = nc.dram_tensor("b2row", [1, VC], FP32, kind="ExternalInput").ap()
    sumexp_out = nc.dram_tensor(
        "sumexp_out", [128, n_sub], FP32, kind="ExternalOutput"
    ).ap()
    tgt_out = nc.dram_tensor("tgt_out", [1, NT], FP32, kind="ExternalOutput").ap()

    with contextlib.ExitStack() as ex:
        tc = ex.enter_context(tile.TileContext(nc))
        # few pools: each pool costs per-engine barrier/sem teardown at exit
        persist_pool = ex.enter_context(tc.tile_pool(name="persist", bufs=1))
        stream_pool = ex.enter_context(tc.tile_pool(name="stream", bufs=2))
        w2_pool = ex.enter_context(tc.tile_pool(name="w2", bufs=4))
        psa_pool = ex.enter_context(tc.tile_pool(name="psa", bufs=3, space="PSUM"))
        pstp_pool = ex.enter_context(tc.tile_pool(name="pstp", bufs=1, space="PSUM"))
        ps2_pool = ex.enter_context(tc.tile_pool(name="ps2", bufs=2, space="PSUM"))
        const_pool = w1_pool = emb_pool = acc_pool = persist_pool
        ctx_pool = tgtw_pool = tmp_pool = stream_pool
        ps1_pool = psa_pool
        pst_pool = pstp_pool

        w2v = w2t.rearrange("(e p) v -> p e v", p=128)
        tgv = w2tgtT.rearrange("(e p) t -> p e t", p=128)
        ctxv = ctxT.rearrange("(k p) t -> p k t", p=128)

        def issue_ctx(bi):
            # two independent tiles so the two halves transfer concurrently
            # (multiple DMAs into one tile serialize on write-after-write)
            bs, bn = cfg.tblocks[bi]
            hk = n_k // 2
            ta = ctx_pool.tile([128, hk, 512], FP8, tag="ctxa")
            tb = ctx_pool.tile([128, hk, 512], FP8, tag="ctxb")
            nc.sync.dma_start(ta[:, :, :bn], ctxv[:, :hk, bs : bs + bn])
            nc.scalar.dma_start(tb[:, :, :bn], ctxv[:, hk:, bs : bs + bn])
            return ta, tb

        def issue_tgw(bi):
            bs, bn = cfg.tblocks[bi]
            t = tgtw_pool.tile([128, n_e * 512], BF16, tag="tgw")
            nc.gpsimd.dma_start(t[:, : n_e * bn], tgv[:, :, bs : bs + bn])
            return t

        def issue_w2(vi):
            vo, vn = cfg.vblocks[vi]
            t = w2_pool.tile([128, n_e, 1024], FP8, tag="w2s")
            nc.sync.dma_start(t[:, :, :vn], w2v[:, :, vo : vo + vn])
            return t

        # ---- PE warm-up: junk matmuls on a memset tile so the HAM clock
        # gate is at 8/8 by the time the real (DMA-gated) matmuls start ----
        WRM = stream_pool.tile([128, 512], BF16, tag="warm")
        nc.vector.memset(WRM[:, :], 0.0)
        psw = ps1_pool.tile([128, 512], FP32, tag="ps1")
        for i in range(14):
            nc.tensor.matmul(psw[:, :], WRM[:, :128], WRM[:, :], start=True, stop=True)

        # ---- upfront DMA issue, in consumption order: only ~8 DMA
        # completion-sem lanes exist and a lane frees when the consumer
        # instruction runs, so issue what phase A needs first and keep
        # everything else off the two HWDGE engines (memset / gpsimd)
        ONES = const_pool.tile([128, 1], BF16, tag="ones")
        nc.gpsimd.memset(ONES[:, :], 1.0)
        B1S = const_pool.tile([128, n_e], FP32, tag="b1s")
        if cfg.use_b1:
            nc.gpsimd.dma_start(B1S[:, :], b1.rearrange("(e p) one -> p (e one)", p=128))
        else:
            nc.gpsimd.memset(B1S[:, :], 0.0)
        ctx_tiles = {0: issue_ctx(0)}
        w1_tiles = []
        for e in range(n_e):
            t = w1_pool.tile([128, n_k, 128], FP8, tag=f"w1s{e}")
            eng = nc.sync if e % 2 == 0 else nc.scalar
            eng.dma_start(t[:, :, :], w1t[:, e * n_k * 128 : (e + 1) * n_k * 128])
            w1_tiles.append(t)
        tgw_tiles = {0: issue_tgw(0)}
        if len(cfg.tblocks) > 1:
            ctx_tiles[1] = issue_ctx(1)
        if cfg.use_b2:
            B2S = const_pool.tile([1, VC], FP32, tag="b2s")
            nc.gpsimd.dma_start(B2S[:, :], b2row[:, :])
            ONE1 = const_pool.tile([1, 128], FP32, tag="one1")
            nc.any.memset(ONE1[:, :], 1.0)
        w2_tiles = {0: issue_w2(0)}
        if n_vb > 1:
            w2_tiles[1] = issue_w2(1)

        EMB8 = emb_pool.tile([128, n_e, NT], FP8, tag="emb8")
        SUMP = acc_pool.tile([128, n_sub * n_vb], FP32, tag="sump")
        SOUT = acc_pool.tile([128, n_sub], FP32, tag="sout")
        TGT = acc_pool.tile([1, NT], FP32, tag="tgt")

        # ---- phase A: emb = sigmoid(W1 @ ctx + b1), [e, t] layout ----
        sig_scale = 1.0 / W1_SCALE
        hk2 = n_k // 4  # kp pairs per ctx half
        for bi, (bs, bn) in enumerate(cfg.tblocks):
            CTA, CTB = ctx_tiles.pop(bi) if bi in ctx_tiles else issue_ctx(bi)
            for e in range(n_e):
                ps1 = ps1_pool.tile([128, 512], FP32, tag="ps1")
                for kp in range(n_k // 2):
                    ct = CTA if kp < hk2 else CTB
                    ko = 2 * kp if kp < hk2 else 2 * (kp - hk2)
                    nc.tensor.matmul(
                        ps1[:, :bn],
                        w1_tiles[e][:, 2 * kp : 2 * kp + 2, :],
                        ct[:, ko : ko + 2, :bn],
                        start=(kp == 0),
                        stop=(kp == n_k // 2 - 1),
                        perf_mode=mybir.MatmulPerfMode.DoubleRow,
                    )
                nc.scalar.activation(
                    EMB8[:, e : e + 1, bs : bs + bn],
                    ps1[:, :bn],
                    AF.Sigmoid,
                    bias=B1S[:, e : e + 1],
                    scale=sig_scale,
                )
            # ---- phase A2: tgt_raw for this block ----
            TGW = tgw_tiles.pop(bi) if bi in tgw_tiles else issue_tgw(bi)
            if bi + 2 < len(cfg.tblocks):
                ctx_tiles[bi + 2] = issue_ctx(bi + 2)
            pst = pst_pool.tile([1, 512], FP32, tag="pst")
            for e in range(n_e):
                tmp = tmp_pool.tile([128, 512], BF16, tag="tmp")
                eng = nc.vector if e % 2 == 0 else nc.gpsimd
                eng.tensor_mul(
                    tmp[:, :bn],
                    EMB8[:, e, bs : bs + bn],
                    TGW[:, e * bn : (e + 1) * bn],
                )
                nc.tensor.matmul(
                    pst[:, :bn],
                    ONES[:, :],
                    tmp[:, :bn],
                    start=(e == 0),
                    stop=(e == n_e - 1),
                )
            nc.vector.tensor_copy(TGT[:, bs : bs + bn], pst[:, :bn])
        nc.gpsimd.dma_start(tgt_out[:, :], TGT[:, :])

        # ---- phase B: logits, exp, accumulate ----
        exp_scale = cfg.inv_temp / W2_SCALE
        for vi, (vo, vn) in enumerate(cfg.vblocks):
            W2S8 = w2_tiles.pop(vi) if vi in w2_tiles else issue_w2(vi)
            if vi == 0:
                for j in (2, 3):
                    if j < n_vb:
                        w2_tiles[j] = issue_w2(j)
            elif vi + 3 < n_vb:
                w2_tiles[vi + 3] = issue_w2(vi + 3)
            for sub in range(n_sub):
                ps2 = ps2_pool.tile([128, 1024], FP32, tag="ps2")
                for ep in range(n_e // 2):
                    lhsT = EMB8[:, 2 * ep : 2 * ep + 2, sub * 128 : (sub + 1) * 128]
                    for h in range(vn // 512):
                        nc.tensor.matmul(
                            ps2[:, h * 512 : (h + 1) * 512],
                            lhsT,
                            W2S8[:, 2 * ep : 2 * ep + 2, h * 512 : (h + 1) * 512],
                            start=(ep == 0),
                            stop=(ep == n_e // 2 - 1) and not cfg.use_b2,
                            perf_mode=mybir.MatmulPerfMode.DoubleRow,
                        )
                if cfg.use_b2:
                    for h in range(vn // 512):
                        nc.tensor.matmul(
                            ps2[:, h * 512 : (h + 1) * 512],
                            ONE1[:, :],
                            B2S[:, vo + h * 512 : vo + (h + 1) * 512],
                            start=False,
                            stop=True,
                        )
                nc.scalar.activation(
                    ps2[:, :vn],
                    ps2[:, :vn],
                    AF.Exp,
                    scale=exp_scale,
                    accum_out=SUMP[:, sub * n_vb + vi : sub * n_vb + vi + 1],
                )

        # ---- phase C: reduce partials, write outputs ----
        for sub in range(n_sub):
            nc.vector.reduce_sum(
                SOUT[:, sub : sub + 1],
                SUMP[:, sub * n_vb : (sub + 1) * n_vb],
                axis=mybir.AxisListType.X,
            )
        nc.sync.dma_start(sumexp_out[:, :], SOUT[:, :])

    nc.compile()
    return nc


# ---------------- host side ----------------


def _plan(lens):
    """Compact valid (b-major) token indices; deal them across NC cores."""
    lens = np.asarray(lens, np.int64)
    nv = np.clip(lens - 2, 0, T - 2)
    n_valid = int(nv.sum())
    NT = max(128, 128 * int(np.ceil(n_valid / (NC * 128))))
    base = np.arange(T - 2, dtype=np.int64)
    idx_all = np.concatenate([b * (T - 2) + base[: nv[b]] for b in range(B)])
    idx = np.zeros(NC * NT, np.int64)
    idx[:n_valid] = idx_all
    idx = idx.reshape(NC, NT)
    counts = [max(0, min(NT, n_valid - c * NT)) for c in range(NC)]
    return NT, idx, counts


def _shard_inputs(hidden, token, W1, W2, NT, idx, b1=None):
    bf16 = ml_dtypes.bfloat16
    half = H // 2
    ctx = np.concatenate(
        [hidden[: T - 2, :, :half], hidden[2:, :, half:]], axis=-1
    )  # [T-2, B, H]
    ctxT8 = np.transpose(ctx, (2, 1, 0)).reshape(H, B * (T - 2)).astype(FP8NP)
    # [p, e, k, ec] = W1[e*128+ec, k*128+p] -> flat [128, n_k*E]
    n_e, n_k = E // 128, H // 128
    W1T = np.ascontiguousarray(
        (W1.reshape(n_e, 128, n_k, 128) * W1_SCALE).transpose(3, 0, 2, 1)
    ).reshape(128, n_k * E).astype(FP8NP)
    W2T = np.zeros((E, VC), dtype=FP8NP)
    W2T[:, :V] = (W2.T * W2_SCALE).astype(FP8NP)
    tgtf = np.ascontiguousarray(token[1 : T - 1].T).reshape(B * (T - 2))

    in_maps, tgt_ids = [], []
    for c in range(NC):
        ic = idx[c]
        tg = tgtf[ic]  # [NT] target token id per slot
        w2gT = np.ascontiguousarray(W2[tg, :].T).astype(bf16)  # [E, NT]
        tgt_ids.append(tg)
        in_maps.append(
            dict(
                ctxT=np.ascontiguousarray(ctxT8[:, ic]),
                w1t=W1T,
                w2t=W2T,
                w2tgtT=w2gT,
            )
        )
    return in_maps, tgt_ids


def _combine(results, counts, tgt_ids, b2, inv_temp, NT):
    """results: list of NC dicts with sumexp_out [128, n_sub], tgt_out [1, NT]."""
    it = float(np.asarray(inv_temp).reshape(-1)[0])
    b2 = np.asarray(b2, dtype=np.float64)
    total_nll = 0.0
    total_cnt = 0
    for c in range(NC):
        k = counts[c]
        if k == 0:
            continue
        se = np.asarray(results[c]["sumexp_out"], dtype=np.float64)  # [128, n_sub]
        S = se.T.reshape(NT)[:k] - PAD_V  # exp(0)=1 per zero-padded vocab col
        raw = np.asarray(results[c]["tgt_out"], dtype=np.float64).reshape(NT)[:k]
        tg = tgt_ids[c][:k]
        logp_tgt = (raw + b2[tg]) * it - np.log(S)
        total_nll += -logp_tgt.sum()
        total_cnt += k
    return np.float32(total_nll / total_cnt)


def kernel(hidden, lens, token, W1, b1, W2, b2, inv_temp):
    hidden = np.asarray(hidden, dtype=np.float32)
    lens = np.asarray(lens, dtype=np.int32)
    token = np.asarray(token, dtype=np.int32)
    W1 = np.asarray(W1, dtype=np.float32)
    b1 = np.asarray(b1, dtype=np.float32)
    W2 = np.asarray(W2, dtype=np.float32)
    b2 = np.asarray(b2, dtype=np.float32)
    inv_temp = np.asarray(inv_temp, dtype=np.float32)

    use_b2 = bool(np.any(b2 != 0.0))
    use_b1 = bool(np.any(b1 != 0.0))
    NT, idx, counts = _plan(lens)
    cfg = Cfg(H, E, NT, VC, inv_temp=float(inv_temp.reshape(-1)[0]), use_b2=use_b2,
              use_b1=use_b1)
    nc = build_lm_program(cfg)
    in_maps, tgt_ids = _shard_inputs(hidden, token, W1, W2, NT, idx,
                                     b1=b1 if use_b1 else None)
    if use_b1:
        b1c = np.ascontiguousarray(np.asarray(b1).reshape(E, 1)).astype(np.float32)
        for c in range(NC):
            in_maps[c]["b1"] = b1c
    if use_b2:
        b2p = np.zeros((1, VC), dtype=np.float32)
        b2p[0, :V] = b2 * W2_SCALE
        for c in range(NC):
            in_maps[c]["b2row"] = b2p
    res = run_bass_kernel_spmd(nc, in_maps, core_ids=list(range(NC)))
    return _combine(res.results, counts, tgt_ids, b2, inv_temp, NT)
